# revision 2
# baseline (speedup 1.0000x reference)
"""Depthwise 13x13 stride-4 conv (AntiAliasInterpolation2d) on 8 TRN2 NeuronCores.

Strategy
--------
Pure data parallel: batch 32 -> 4 images per core. On each core the conv is
computed as PSUM-accumulated banded-Toeplitz matmuls on the TensorEngine:

  out[y, x] = sum_{i,j} w[i,j] * in[4y + i - 6, 4x + j - 6]

Contract over input rows r (the SBUF partition dim). For each kernel column j
and each 128-row chunk k of the input, one matmul:

  lhsT = A_jk [128 rows, 128 y]   (banded Toeplitz built from w[:, j], host-built)
  rhs  = in[rows of chunk k, 4x + j - 6]   for 4 images -> N = 4*128 = 512
  psum[y, (img,x)] += lhsT.T @ rhs         (13 j * 4 k = 52 matmuls accumulate)

The stride-4 column access is made contiguous by de-interleaving the (padded)
input columns into 4 phases on the host. All Toeplitz matrices for one
(channel, j) are column-shifted views of a single [128, 130]-slot band buffer
("epack"), so lhsT never needs per-chunk storage. Everything computes in bf16
(fp32 PSUM accumulation), which halves HBM traffic; output is fp32.
"""

import numpy as np
import ml_dtypes

N_CORES = 8
B, C, H, W = 32, 3, 512, 512
KS = 13          # kernel size
PAD = 6          # pad on each side
STR = 4          # stride
OH = OW = 128    # output spatial
PW = W + 2 * PAD  # 524 padded width
NPH = PW // STR   # 131 columns per phase
SLOT = 130        # epack columns per (c, j) pair
NPAIR = C * KS    # 39
EPACK_COLS = (NPAIR - 1) * SLOT + 224  # last window needs 96+128 cols
BPC = B // N_CORES  # images per core = 4

_CACHE = {}


def _build_graph():
    import concourse.bass as bass  # noqa: F401
    import concourse.tile as tile
    from concourse import bacc, mybir

    nc = bacc.Bacc(
        "TRN2", target_bir_lowering=False, debug=False, num_devices=N_CORES
    )
    x = nc.dram_tensor(
        "x", [C, 4, 128, STR * PW], mybir.dt.bfloat16, kind="ExternalInput"
    )
    ep = nc.dram_tensor(
        "ep", [128, EPACK_COLS], mybir.dt.bfloat16, kind="ExternalInput"
    )
    out = nc.dram_tensor(
        "out", [BPC, C, OH, OW], mybir.dt.float32, kind="ExternalOutput"
    )

    with tile.TileContext(nc) as tc:
        with (
            tc.tile_pool(name="const", bufs=1) as constp,
            tc.tile_pool(name="xin", bufs=4) as xin,
            tc.tile_pool(name="ps", bufs=2, space="PSUM") as psp,
            tc.tile_pool(name="ot", bufs=2) as otp,
        ):
            ept = constp.tile([128, EPACK_COLS], mybir.dt.bfloat16)
            nc.sync.dma_start(ept[:], ep[:])
            for c in range(C):
                psum = psp.tile([128, BPC * OW], mybir.dt.float32)
                for k in range(4):
                    xt = xin.tile([128, STR * PW], mybir.dt.bfloat16)
                    nc.sync.dma_start(xt[:], x[c, k])
                    # [128 rows, img 4, phase*131 + u]
                    xg = xt[:].rearrange("p (g w) -> p g w", g=BPC)
                    for j in range(KS):
                        ph, q = j % STR, j // STR
                        off = ph * NPH + q
                        rhs = xg[:, :, off : off + OW]  # [128, 4, 128]
                        t = c * KS + j
                        lo = t * SLOT + 96 - 32 * k
                        lhsT = ept[:, lo : lo + 128]
                        nc.tensor.matmul(
                            psum[:],
                            lhsT,
                            rhs,
                            start=(k == 0 and j == 0),
                            stop=(k == 3 and j == KS - 1),
                        )
                o = otp.tile([128, BPC * OW], mybir.dt.float32)
                nc.vector.tensor_copy(o[:], psum[:])
                dst = out[:, c].rearrange("g y x -> y g x")
                nc.sync.dma_start(dst, o[:].rearrange("y (g x) -> y g x", g=BPC))
    nc.finalize()
    return nc


def _prep_inputs(inp, weight):
    bf16 = ml_dtypes.bfloat16
    inp = np.asarray(inp, dtype=np.float32)
    weight = np.asarray(weight, dtype=np.float32)

    # pad columns, de-interleave into 4 phases: [b, c, r, phase, u]
    pad = np.zeros((B, C, H, PW), np.float32)
    pad[..., PAD : PAD + W] = inp
    phmat = pad.reshape(B, C, H, NPH, STR).transpose(0, 1, 2, 4, 3)
    # -> [core, c, k, r, img, phase, u] -> [core, c, k, 128, 4*524]
    arr = phmat.reshape(N_CORES, BPC, C, 4, 128, STR, NPH)
    arr = arr.transpose(0, 2, 3, 4, 1, 5, 6).reshape(
        N_CORES, C, 4, 128, STR * PW
    )
    arr = np.ascontiguousarray(arr).astype(bf16)

    # epack: banded Toeplitz slots, one per (c, j)
    epk = np.zeros((128, EPACK_COLS), np.float32)
    r = np.arange(128)
    for c in range(C):
        for j in range(KS):
            t = c * KS + j
            for s in range(-2, 34):
                i = r - 4 * s + PAD  # tap index per row
                m = (i >= 0) & (i < KS)
                if m.any():
                    epk[m, t * SLOT + 96 + s] = weight[c, 0, i[m], j]
    epk = epk.astype(bf16)

    in_maps = [{"x": arr[core], "ep": epk} for core in range(N_CORES)]
    return in_maps


def kernel(inp, weight):
    from concourse.bass_utils import run_bass_kernel_spmd

    if "nc" not in _CACHE:
        _CACHE["nc"] = _build_graph()
    nc = _CACHE["nc"]

    in_maps = _prep_inputs(inp, weight)
    res = run_bass_kernel_spmd(nc, in_maps, core_ids=list(range(N_CORES)))
    outs = [res.results[i]["out"] for i in range(N_CORES)]
    return np.concatenate(outs, axis=0)


# revision 8
# speedup vs baseline: 1.1631x; 1.1631x over previous
"""Depthwise 13x13 stride-4 conv (AntiAliasInterpolation2d) on 8 TRN2 NeuronCores.

Pure data parallel: batch 32 -> 4 images per core. Two device graphs:

1. rank-1 path (used when each channel's 13x13 kernel is an outer product
   v ⊗ h, which holds for the Gaussian anti-alias kernel): separable conv.
   Stage V contracts input rows on the TensorEngine via banded-Toeplitz
   stationaries (stride-4 vertical conv, fp32 PSUM accumulate); the DVE
   copies V to SBUF in bf16 while de-interleaving columns into 4 phases;
   stage H applies the horizontal taps as diagonal-stationary matmuls,
   one per tap, accumulating in PSUM (the stride-4 column gather becomes
   a contiguous slice in phase space).

2. general path (fallback for non-separable weights): direct 2D conv as
   52 PSUM-accumulated banded-Toeplitz matmuls per channel (13 kernel
   columns x 4 row chunks), stride-4 columns de-interleaved on the host.

Everything computes in bf16 (fp32 accumulation); output is fp32.
"""

import numpy as np
import ml_dtypes

N_CORES = 8
B, C, H, W = 32, 3, 512, 512
KS = 13          # kernel size
PAD = 6          # pad on each side
STR = 4          # stride
OH = OW = 128    # output spatial
PW = W + 2 * PAD  # 524 padded width
NPH = PW // STR   # 131 columns per phase
BPC = B // N_CORES  # images per core = 4
XW = BPC * PW     # 2096 free-dim columns per input tile

# general path epack layout
SLOT = 130
NPAIR = C * KS
EPACK_COLS = (NPAIR - 1) * SLOT + 224

_CACHE = {}


def _bacc():
    from concourse import bacc

    return bacc.Bacc(
        "TRN2", target_bir_lowering=False, debug=False, num_devices=N_CORES
    )


def _build_graph_rank1():
    import concourse.tile as tile
    from concourse import mybir

    nc = _bacc()
    # input: [c, k(row chunk), 128 rows, img*524 padded cols] bf16
    x = nc.dram_tensor("x", [C, 4, 128, XW], mybir.dt.bfloat16, kind="ExternalInput")
    # per-channel stationaries: 4 vertical Toeplitz chunks + 13 diag taps
    # (host packs partition-major so the DMA is contiguous per partition)
    av = nc.dram_tensor("av", [C, 128, 4 * 128], mybir.dt.bfloat16, kind="ExternalInput")
    dg = nc.dram_tensor("dg", [C, 128, KS * 128], mybir.dt.bfloat16, kind="ExternalInput")
    out = nc.dram_tensor("out", [BPC, C, OH, OW], mybir.dt.float32, kind="ExternalOutput")

    f32 = mybir.dt.float32
    bf16 = mybir.dt.bfloat16

    with tile.TileContext(nc) as tc:
        with (
            tc.tile_pool(name="avp", bufs=2) as avp,
            tc.tile_pool(name="dgp", bufs=2) as dgp,
            tc.tile_pool(name="xin", bufs=8) as xin,
            tc.tile_pool(name="vsb", bufs=2) as vsbp,
            tc.tile_pool(name="vpsA", bufs=4, space="PSUM") as vpsA,
            tc.tile_pool(name="vpsB", bufs=2, space="PSUM") as vpsB,
            tc.tile_pool(name="hps", bufs=2, space="PSUM") as hps,
            tc.tile_pool(name="ot", bufs=2) as otp,
        ):
            for c in range(C):
                avt = avp.tile([128, 4 * 128], bf16)
                nc.scalar.dma_start(avt[:], av[c])
                dgt = dgp.tile([128, KS * 128], bf16)
                nc.scalar.dma_start(dgt[:], dg[c])

                # ---- stage V: vertical 13-tap stride-4 conv via Toeplitz ----
                vA = [
                    vpsA.tile([128, 512], f32, tag="vA", name=f"vA_{c}_{g}")
                    for g in range(BPC)
                ]
                vB = vpsB.tile([128, BPC * 12], f32)
                xts = []
                for k in range(4):
                    xt = xin.tile([128, XW], bf16)
                    nc.sync.dma_start(xt[:], x[c, k])
                    xts.append(xt)
                for k in range(4):
                    xt = xts[k]
                    lhsT = avt[:, k * 128 : (k + 1) * 128]
                    for g in range(BPC):
                        nc.tensor.matmul(
                            vA[g][:],
                            lhsT,
                            xt[:, g * PW : g * PW + 512],
                            start=(k == 0),
                            stop=(k == 3),
                        )
                    rhsB = xt[:].rearrange("p (g w) -> p g w", g=BPC)[:, :, 512:524]
                    nc.tensor.matmul(vB[:], lhsT, rhsB, start=(k == 0), stop=(k == 3))

                # ---- copy V to SBUF bf16, de-interleaving into phases ----
                # vsb free layout per image: [phase 4][u 131]
                vsb = vsbp.tile([128, XW], bf16)
                for g in range(BPC):
                    dstA = vsb[:].rearrange(
                        "p (g ph u) -> p g ph u", g=BPC, ph=STR
                    )[:, g, :, 0:128]
                    srcA = vA[g][:].rearrange("p (u ph) -> p ph u", ph=STR)
                    nc.vector.tensor_copy(dstA, srcA)
                dstB = vsb[:].rearrange("p (g ph u) -> p g ph u", g=BPC, ph=STR)[
                    :, :, :, 128:131
                ]
                srcB = vB[:].rearrange("p (g u ph) -> p g ph u", g=BPC, ph=STR)
                nc.vector.tensor_copy(dstB, srcB)

                # ---- stage H: 13 taps as diagonal-stationary matmuls ----
                hp = hps.tile([128, BPC * OW], f32)
                vg = vsb[:].rearrange("p (g w) -> p g w", g=BPC)
                for j in range(KS):
                    ph, q = j % STR, j // STR
                    off = ph * NPH + q
                    rhs = vg[:, :, off : off + OW]
                    lhsT = dgt[:, j * 128 : (j + 1) * 128]
                    nc.tensor.matmul(
                        hp[:], lhsT, rhs, start=(j == 0), stop=(j == KS - 1)
                    )

                o = otp.tile([128, BPC * OW], f32)
                nc.vector.tensor_copy(o[:], hp[:])
                dst = out[:, c].rearrange("g y x -> y g x")
                nc.sync.dma_start(dst, o[:].rearrange("y (g x) -> y g x", g=BPC))
    nc.finalize()
    return nc


def _build_graph_general():
    import concourse.tile as tile
    from concourse import mybir

    nc = _bacc()
    x = nc.dram_tensor("x", [C, 4, 128, XW], mybir.dt.bfloat16, kind="ExternalInput")
    ep = nc.dram_tensor("ep", [128, EPACK_COLS], mybir.dt.bfloat16, kind="ExternalInput")
    out = nc.dram_tensor("out", [BPC, C, OH, OW], mybir.dt.float32, kind="ExternalOutput")

    with tile.TileContext(nc) as tc:
        with (
            tc.tile_pool(name="const", bufs=1) as constp,
            tc.tile_pool(name="xin", bufs=4) as xin,
            tc.tile_pool(name="ps", bufs=2, space="PSUM") as psp,
            tc.tile_pool(name="ot", bufs=2) as otp,
        ):
            ept = constp.tile([128, EPACK_COLS], mybir.dt.bfloat16)
            nc.scalar.dma_start(ept[:], ep[:])
            for c in range(C):
                psum = psp.tile([128, BPC * OW], mybir.dt.float32)
                for k in range(4):
                    xt = xin.tile([128, XW], mybir.dt.bfloat16)
                    nc.sync.dma_start(xt[:], x[c, k])
                    xg = xt[:].rearrange("p (g w) -> p g w", g=BPC)
                    for j in range(KS):
                        ph, q = j % STR, j // STR
                        off = ph * NPH + q
                        rhs = xg[:, :, off : off + OW]
                        t = c * KS + j
                        lo = t * SLOT + 96 - 32 * k
                        lhsT = ept[:, lo : lo + 128]
                        nc.tensor.matmul(
                            psum[:],
                            lhsT,
                            rhs,
                            start=(k == 0 and j == 0),
                            stop=(k == 3 and j == KS - 1),
                        )
                o = otp.tile([128, BPC * OW], mybir.dt.float32)
                nc.vector.tensor_copy(o[:], psum[:])
                dst = out[:, c].rearrange("g y x -> y g x")
                nc.sync.dma_start(dst, o[:].rearrange("y (g x) -> y g x", g=BPC))
    nc.finalize()
    return nc


def _decompose(weight):
    """Per-channel SVD; return (v[c,13], h[c,13]) if rank-1, else None."""
    vs, hs = [], []
    for c in range(C):
        w = weight[c, 0].astype(np.float64)
        u, s, vt = np.linalg.svd(w)
        if s[1] > 1e-5 * s[0]:
            return None
        sc = np.sqrt(s[0])
        vs.append(u[:, 0] * sc)
        hs.append(vt[0] * sc)
    return np.stack(vs), np.stack(hs)


def _pad_shard(inp):
    """[32,3,512,512] f32 -> [core, c, k, 128, img*524] bf16 (padded cols)."""
    bf16 = ml_dtypes.bfloat16
    pad = np.zeros((B, C, H, PW), np.float32)
    pad[..., PAD : PAD + W] = inp
    arr = pad.reshape(N_CORES, BPC, C, 4, 128, PW)
    arr = arr.transpose(0, 2, 3, 4, 1, 5).reshape(N_CORES, C, 4, 128, XW)
    return np.ascontiguousarray(arr).astype(bf16)


def _phase_shard(inp):
    """[32,3,512,512] f32 -> padded + phase-deinterleaved shards (general)."""
    bf16 = ml_dtypes.bfloat16
    pad = np.zeros((B, C, H, PW), np.float32)
    pad[..., PAD : PAD + W] = inp
    phmat = pad.reshape(B, C, H, NPH, STR).transpose(0, 1, 2, 4, 3)
    arr = phmat.reshape(N_CORES, BPC, C, 4, 128, STR, NPH)
    arr = arr.transpose(0, 2, 3, 4, 1, 5, 6).reshape(N_CORES, C, 4, 128, XW)
    return np.ascontiguousarray(arr).astype(bf16)


def _toeplitz_band(vec):
    """[13] taps -> [4, 128, 128] vertical Toeplitz chunks A[k][r, y]."""
    a = np.zeros((4, 128, 128), np.float32)
    for k in range(4):
        r = np.arange(128)[:, None] + 128 * k
        y = np.arange(128)[None, :]
        i = r - 4 * y + PAD
        m = (i >= 0) & (i < KS)
        a[k][m] = vec[i[m]]
    return a


def _prep_rank1(inp, v, h):
    bf16 = ml_dtypes.bfloat16
    arr = _pad_shard(inp)
    # av[c, p, k*128+m] = Toeplitz chunk k at [p, m]
    av = np.stack(
        [_toeplitz_band(v[c]).transpose(1, 0, 2).reshape(128, 4 * 128) for c in range(C)]
    ).astype(bf16)
    eye = np.eye(128, dtype=np.float32)
    # dg[c, p, j*128+m] = h[c, j] * I
    dg = np.stack(
        [
            np.stack([h[c, j] * eye for j in range(KS)])
            .transpose(1, 0, 2)
            .reshape(128, KS * 128)
            for c in range(C)
        ]
    ).astype(bf16)
    return [{"x": arr[core], "av": av, "dg": dg} for core in range(N_CORES)]


def _prep_general(inp, weight):
    bf16 = ml_dtypes.bfloat16
    arr = _phase_shard(inp)
    epk = np.zeros((128, EPACK_COLS), np.float32)
    r = np.arange(128)
    for c in range(C):
        for j in range(KS):
            t = c * KS + j
            for s in range(-2, 34):
                i = r - 4 * s + PAD
                m = (i >= 0) & (i < KS)
                if m.any():
                    epk[m, t * SLOT + 96 + s] = weight[c, 0, i[m], j]
    epk = epk.astype(bf16)
    return [{"x": arr[core], "ep": epk} for core in range(N_CORES)]


def _prep(inp, weight):
    """Returns (graph_key, in_maps)."""
    inp = np.asarray(inp, dtype=np.float32)
    weight = np.asarray(weight, dtype=np.float32)
    vh = _decompose(weight)
    if vh is not None:
        return "rank1", _prep_rank1(inp, *vh)
    return "general", _prep_general(inp, weight)


def _graph(key):
    if key not in _CACHE:
        _CACHE[key] = (
            _build_graph_rank1() if key == "rank1" else _build_graph_general()
        )
    return _CACHE[key]


def kernel(inp, weight):
    from concourse.bass_utils import run_bass_kernel_spmd

    key, in_maps = _prep(inp, weight)
    nc = _graph(key)
    res = run_bass_kernel_spmd(nc, in_maps, core_ids=list(range(N_CORES)))
    outs = [res.results[i]["out"] for i in range(N_CORES)]
    return np.concatenate(outs, axis=0)


# revision 12
# speedup vs baseline: 1.2134x; 1.0433x over previous
"""Depthwise 13x13 stride-4 conv (AntiAliasInterpolation2d) on 8 TRN2 NeuronCores.

Pure data parallel: batch 32 -> 4 images per core. Two device graphs:

1. rank-1 path (used when each channel's 13x13 kernel is an outer product
   v ⊗ h, which holds for the Gaussian anti-alias kernel): separable conv.
   Stage V contracts input rows on the TensorEngine via banded-Toeplitz
   stationaries (stride-4 vertical conv, fp32 PSUM accumulate); the DVE
   copies V to SBUF in bf16 while de-interleaving columns into 4 phases;
   stage H applies the horizontal taps as diagonal-stationary matmuls,
   one per tap, accumulating in PSUM (the stride-4 column gather becomes
   a contiguous slice in phase space).

2. general path (fallback for non-separable weights): direct 2D conv as
   52 PSUM-accumulated banded-Toeplitz matmuls per channel (13 kernel
   columns x 4 row chunks), stride-4 columns de-interleaved on the host.

Everything computes in bf16 (fp32 accumulation); output is fp32.
"""

import numpy as np
import ml_dtypes

N_CORES = 8
B, C, H, W = 32, 3, 512, 512
KS = 13          # kernel size
PAD = 6          # pad on each side
STR = 4          # stride
OH = OW = 128    # output spatial
PW = W + 2 * PAD  # 524 padded width
NPH = PW // STR   # 131 columns per phase
BPC = B // N_CORES  # images per core = 4
XW = BPC * PW     # 2096 free-dim columns per input tile

# general path epack layout
SLOT = 130
NPAIR = C * KS
EPACK_COLS = (NPAIR - 1) * SLOT + 224

_CACHE = {}


def _bacc():
    from concourse import bacc

    return bacc.Bacc(
        "TRN2", target_bir_lowering=False, debug=False, num_devices=N_CORES
    )


STCOLS = 4 * 128 + KS * 32  # per-channel stationaries: 4 Toeplitz + 13 diag32


def _build_graph_rank1():
    import concourse.tile as tile
    from concourse import mybir

    nc = _bacc()
    # input: [c, 128 rows, k(row chunk) * img * 524 padded cols] bf16
    x = nc.dram_tensor("x", [C, 128, 4 * XW], mybir.dt.bfloat16, kind="ExternalInput")
    # per-channel stationaries, partition-major: [av: 4*128 | diag32: 13*32]
    st = nc.dram_tensor("st", [C, 128, STCOLS], mybir.dt.bfloat16, kind="ExternalInput")
    out = nc.dram_tensor("out", [BPC, C, OH, OW], mybir.dt.float32, kind="ExternalOutput")

    f32 = mybir.dt.float32
    bf16 = mybir.dt.bfloat16

    with tile.TileContext(nc) as tc:
        with (
            tc.tile_pool(name="stp", bufs=3) as stp,
            tc.tile_pool(name="xin", bufs=3) as xin,
            tc.tile_pool(name="vsb", bufs=2) as vsbp,
            tc.tile_pool(name="vpsA", bufs=3, space="PSUM") as vpsA,
            tc.tile_pool(name="vpsB", bufs=2, space="PSUM") as vpsB,
            tc.tile_pool(name="hps", bufs=2, space="PSUM") as hps,
            tc.tile_pool(name="ot", bufs=2) as otp,
        ):
            for c in range(C):
                stt = stp.tile([128, STCOLS], bf16)
                nc.scalar.dma_start(stt[:], st[c])
                xt = xin.tile([128, 4 * XW], bf16)
                nc.sync.dma_start(xt[:], x[c])

                # ---- stage V: vertical 13-tap stride-4 conv via Toeplitz ----
                # x free layout: [k 4][img 4][524]
                vsb = vsbp.tile([128, XW], bf16)
                vB = vpsB.tile([128, BPC * 12], f32)
                vgall = vsb[:].rearrange("p (g ph u) -> p g ph u", g=BPC, ph=STR)
                for g in range(BPC):
                    vA = vpsA.tile([128, 512], f32, tag="vA", name=f"vA_{c}_{g}")
                    for k in range(4):
                        nc.tensor.matmul(
                            vA[:],
                            stt[:, k * 128 : (k + 1) * 128],
                            xt[:, (k * BPC + g) * PW : (k * BPC + g) * PW + 512],
                            start=(k == 0),
                            stop=(k == 3),
                        )
                    # copy V to SBUF bf16, de-interleaving into 4 phases
                    srcA = vA[:].rearrange("p (u ph) -> p ph u", ph=STR)
                    nc.vector.tensor_copy(vgall[:, g, :, 0:128], srcA)
                # rightmost 12 padded columns of each image, all images at once
                xg = xt[:].rearrange("p (k g w) -> p k g w", k=4, g=BPC)
                for k in range(4):
                    nc.tensor.matmul(
                        vB[:],
                        stt[:, k * 128 : (k + 1) * 128],
                        xg[:, k, :, 512:524],
                        start=(k == 0),
                        stop=(k == 3),
                    )
                srcB = vB[:].rearrange("p (g u ph) -> p g ph u", g=BPC, ph=STR)
                nc.vector.tensor_copy(vgall[:, :, :, 128:131], srcB)

                # ---- stage H: 13 taps, diag32 stationaries col-tiled 4x ----
                hp = hps.tile([128, BPC * OW], f32)
                vg = vsb[:].rearrange("p (g w) -> p g w", g=BPC)
                for j in range(KS):
                    ph, q = j % STR, j // STR
                    off = ph * NPH + q
                    for i in range(4):
                        nc.tensor.matmul(
                            hp[32 * i : 32 * i + 32, :],
                            stt[32 * i : 32 * i + 32, 512 + j * 32 : 512 + j * 32 + 32],
                            vg[32 * i : 32 * i + 32, :, off : off + OW],
                            start=(j == 0),
                            stop=(j == KS - 1),
                            skip_group_check=True,
                            tile_position=(32 * i, 32 * i),
                        )

                o = otp.tile([128, BPC * OW], f32)
                nc.vector.tensor_copy(o[:], hp[:])
                dst = out[:, c].rearrange("g y x -> y g x")
                nc.sync.dma_start(dst, o[:].rearrange("y (g x) -> y g x", g=BPC))
    nc.finalize()
    return nc


def _build_graph_general():
    import concourse.tile as tile
    from concourse import mybir

    nc = _bacc()
    x = nc.dram_tensor("x", [C, 4, 128, XW], mybir.dt.bfloat16, kind="ExternalInput")
    ep = nc.dram_tensor("ep", [128, EPACK_COLS], mybir.dt.bfloat16, kind="ExternalInput")
    out = nc.dram_tensor("out", [BPC, C, OH, OW], mybir.dt.float32, kind="ExternalOutput")

    with tile.TileContext(nc) as tc:
        with (
            tc.tile_pool(name="const", bufs=1) as constp,
            tc.tile_pool(name="xin", bufs=4) as xin,
            tc.tile_pool(name="ps", bufs=2, space="PSUM") as psp,
            tc.tile_pool(name="ot", bufs=2) as otp,
        ):
            ept = constp.tile([128, EPACK_COLS], mybir.dt.bfloat16)
            nc.scalar.dma_start(ept[:], ep[:])
            for c in range(C):
                psum = psp.tile([128, BPC * OW], mybir.dt.float32)
                for k in range(4):
                    xt = xin.tile([128, XW], mybir.dt.bfloat16)
                    nc.sync.dma_start(xt[:], x[c, k])
                    xg = xt[:].rearrange("p (g w) -> p g w", g=BPC)
                    for j in range(KS):
                        ph, q = j % STR, j // STR
                        off = ph * NPH + q
                        rhs = xg[:, :, off : off + OW]
                        t = c * KS + j
                        lo = t * SLOT + 96 - 32 * k
                        lhsT = ept[:, lo : lo + 128]
                        nc.tensor.matmul(
                            psum[:],
                            lhsT,
                            rhs,
                            start=(k == 0 and j == 0),
                            stop=(k == 3 and j == KS - 1),
                        )
                o = otp.tile([128, BPC * OW], mybir.dt.float32)
                nc.vector.tensor_copy(o[:], psum[:])
                dst = out[:, c].rearrange("g y x -> y g x")
                nc.sync.dma_start(dst, o[:].rearrange("y (g x) -> y g x", g=BPC))
    nc.finalize()
    return nc


def _decompose(weight):
    """Per-channel SVD; return (v[c,13], h[c,13]) if rank-1, else None."""
    vs, hs = [], []
    for c in range(C):
        w = weight[c, 0].astype(np.float64)
        u, s, vt = np.linalg.svd(w)
        if s[1] > 1e-5 * s[0]:
            return None
        sc = np.sqrt(s[0])
        vs.append(u[:, 0] * sc)
        hs.append(vt[0] * sc)
    return np.stack(vs), np.stack(hs)


def _pad_shard(inp):
    """[32,3,512,512] f32 -> [core, c, 128, k*img*524] bf16 (padded cols)."""
    bf16 = ml_dtypes.bfloat16
    pad = np.zeros((B, C, H, PW), np.float32)
    pad[..., PAD : PAD + W] = inp
    arr = pad.reshape(N_CORES, BPC, C, 4, 128, PW)
    arr = arr.transpose(0, 2, 4, 3, 1, 5).reshape(N_CORES, C, 128, 4 * XW)
    return np.ascontiguousarray(arr).astype(bf16)


def _phase_shard(inp):
    """[32,3,512,512] f32 -> padded + phase-deinterleaved shards (general)."""
    bf16 = ml_dtypes.bfloat16
    pad = np.zeros((B, C, H, PW), np.float32)
    pad[..., PAD : PAD + W] = inp
    phmat = pad.reshape(B, C, H, NPH, STR).transpose(0, 1, 2, 4, 3)
    arr = phmat.reshape(N_CORES, BPC, C, 4, 128, STR, NPH)
    arr = arr.transpose(0, 2, 3, 4, 1, 5, 6).reshape(N_CORES, C, 4, 128, XW)
    return np.ascontiguousarray(arr).astype(bf16)


def _toeplitz_band(vec):
    """[13] taps -> [4, 128, 128] vertical Toeplitz chunks A[k][r, y]."""
    a = np.zeros((4, 128, 128), np.float32)
    for k in range(4):
        r = np.arange(128)[:, None] + 128 * k
        y = np.arange(128)[None, :]
        i = r - 4 * y + PAD
        m = (i >= 0) & (i < KS)
        a[k][m] = vec[i[m]]
    return a


def _prep_rank1(inp, v, h):
    bf16 = ml_dtypes.bfloat16
    arr = _pad_shard(inp)
    st = np.zeros((C, 128, STCOLS), np.float32)
    eye32 = np.eye(32, dtype=np.float32)
    for c in range(C):
        # [0, 512): vertical Toeplitz chunks, k-major
        st[c, :, : 4 * 128] = (
            _toeplitz_band(v[c]).transpose(1, 0, 2).reshape(128, 4 * 128)
        )
        # [512, 512+13*32): 32x32 diag h[c, j], replicated down partitions
        for j in range(KS):
            blk = np.tile(h[c, j] * eye32, (4, 1))  # [128, 32]
            st[c, :, 512 + j * 32 : 512 + (j + 1) * 32] = blk
    st = st.astype(bf16)
    return [{"x": arr[core], "st": st} for core in range(N_CORES)]


def _prep_general(inp, weight):
    bf16 = ml_dtypes.bfloat16
    arr = _phase_shard(inp)
    epk = np.zeros((128, EPACK_COLS), np.float32)
    r = np.arange(128)
    for c in range(C):
        for j in range(KS):
            t = c * KS + j
            for s in range(-2, 34):
                i = r - 4 * s + PAD
                m = (i >= 0) & (i < KS)
                if m.any():
                    epk[m, t * SLOT + 96 + s] = weight[c, 0, i[m], j]
    epk = epk.astype(bf16)
    return [{"x": arr[core], "ep": epk} for core in range(N_CORES)]


def _prep(inp, weight):
    """Returns (graph_key, in_maps)."""
    inp = np.asarray(inp, dtype=np.float32)
    weight = np.asarray(weight, dtype=np.float32)
    vh = _decompose(weight)
    if vh is not None:
        return "rank1", _prep_rank1(inp, *vh)
    return "general", _prep_general(inp, weight)


def _graph(key):
    if key not in _CACHE:
        _CACHE[key] = (
            _build_graph_rank1() if key == "rank1" else _build_graph_general()
        )
    return _CACHE[key]


def kernel(inp, weight):
    from concourse.bass_utils import run_bass_kernel_spmd

    key, in_maps = _prep(inp, weight)
    nc = _graph(key)
    res = run_bass_kernel_spmd(nc, in_maps, core_ids=list(range(N_CORES)))
    outs = [res.results[i]["out"] for i in range(N_CORES)]
    return np.concatenate(outs, axis=0)
